# revision 13
# baseline (speedup 1.0000x reference)
"""Trainium2 Bass kernel: LocalEmbeddingLayer (KNN -> gather -> 2-layer GELU MLP -> mean).

Full-input contract: kernel(**inputs) takes the unsharded inputs and returns the
full [B, N, P] output. Internally shards batch B=32 across 8 NeuronCores (pure
data parallel, 4 batch elements per core), runs one SPMD Bass program on all
cores, and concatenates the per-core outputs.

v2 design (vs the ap_gather baseline at 1.84ms):
  - The neighbor-feature gather is a SWDGE dma_gather(transpose=True) from a
    DRAM row table featrow[N, 128] f16 (256B rows, [feat|feat]); the DMA
    engines land it directly in [feat-on-partition, 2048] layout. This
    replaces the ~35us/block ap_gather ucode (which serialized the whole
    pipeline) with a ~2-4us DMA that overlaps compute.
  - The whole MLP runs in f16 (full PE rate like bf16, ~2^-11 element error;
    measured f32r matmuls run ~2.5 cycles/col on HW, f16 ~1).
  - The center-feature term of layer 1 is a second accumulating matmul whose
    rhs is a stride-0 broadcast AP over an SBUF f16 featT tile (16x free-dim
    repeat), so center features are never gathered or materialized.
  - Distance scores stay in the proven bf16 hi/lo split matmul (~fp32
    selection accuracy); top-16 runs on an f16 copy of the scores (DVE 16-bit
    2x mode) with the self-mask at -1e4.
  - Mean over k on the GPSIMD (Pool) engine ucode to unload the DVE.
"""

import numpy as np

B, N, DPOS, F, P, K = 32, 1024, 3, 64, 128, 16
NCORES = 8
BL = B // NCORES          # batches per core
NBLK = N // 128           # row blocks per batch
MASK = 1.0e4              # self-match mask depth (f16-safe)
NEG = -3.0e4              # match_replace purge value (f16-safe)

TOPK_F16 = False          # f16 scans measured no faster on DVE; f32 keeps accuracy
REDUCE_POOL = False       # GPSIMD tensor_reduce can't do free-axis (X) reduces


def build_program(topk_f16=TOPK_F16, reduce_pool=REDUCE_POOL, gelu=True,
                  n_b=BL, n_blk=NBLK):
    import concourse.bacc as bacc
    import concourse.mybir as mybir
    from concourse.tile import TileContext

    f32 = mybir.dt.float32
    f16 = mybir.dt.float16
    bf16 = mybir.dt.bfloat16
    u16 = mybir.dt.uint16
    i16 = mybir.dt.int16
    AF = mybir.ActivationFunctionType
    act_fn = AF.Gelu if gelu else AF.Identity

    nc = bacc.Bacc("TRN2", target_bir_lowering=False)

    ab_d = nc.dram_tensor("ab", [n_b, 2, 16, N], bf16, kind="ExternalInput")
    featTb_d = nc.dram_tensor("featTb", [n_b, 64, N], f16, kind="ExternalInput")
    featrow_d = nc.dram_tensor("featrow", [n_b, N, 128], f16, kind="ExternalInput")
    w1a_d = nc.dram_tensor("w1a", [64, 256], f16, kind="ExternalInput")
    w1c_d = nc.dram_tensor("w1c", [64, 256], f16, kind="ExternalInput")
    w2_d = nc.dram_tensor("w2", [128, 256], f16, kind="ExternalInput")
    b1_d = nc.dram_tensor("b1", [128, 2], f32, kind="ExternalInput")
    b2_d = nc.dram_tensor("b2", [128, 1], f32, kind="ExternalInput")
    cbf_d = nc.dram_tensor("cbf", [128, 256], bf16, kind="ExternalInput")
    out_d = nc.dram_tensor("out", [n_b, n_blk, 128, 128], f32, kind="ExternalOutput")

    with TileContext(nc) as tc:
        with (
            tc.tile_pool(name="const", bufs=1) as cpool,
            tc.tile_pool(name="feat", bufs=2) as fpool,
            tc.tile_pool(name="work", bufs=3) as wpool,
            tc.tile_pool(name="small", bufs=3) as spool,
            tc.tile_pool(name="ps_tk", bufs=1, space="PSUM") as ptk,
            tc.tile_pool(name="ps_h1", bufs=2, space="PSUM") as ph1,
            tc.tile_pool(name="ps_l2", bufs=1, space="PSUM") as pl2,
        ):
            w1a_sb = cpool.tile([64, 256], f16)
            nc.sync.dma_start(w1a_sb[:], w1a_d[:])
            w1c_sb = cpool.tile([64, 256], f16)
            nc.sync.dma_start(w1c_sb[:], w1c_d[:])
            w2_sb = cpool.tile([128, 256], f16)
            nc.sync.dma_start(w2_sb[:], w2_d[:])
            b1_sb = cpool.tile([128, 2], f32)
            nc.sync.dma_start(b1_sb[:], b1_d[:])
            b2_sb = cpool.tile([128, 1], f32)
            nc.sync.dma_start(b2_sb[:], b2_d[:])
            cbf_sb = cpool.tile([128, 256], bf16)   # cols 0:128 I, 128:256 -MASK*I
            nc.sync.dma_start(cbf_sb[:], cbf_d[:])

            feat_tiles = {}

            def stage1(b, blk):
                if blk == 0:
                    ab = fpool.tile([16, 2 * N], bf16, tag="ab")
                    nc.sync.dma_start(
                        ab[:].rearrange("d (x n) -> d x n", x=2),
                        ab_d[b].rearrange("x d n -> d x n"),
                    )
                    ftb = fpool.tile([64, N], f16, tag="featTb")
                    nc.sync.dma_start(ftb[:], featTb_d[b])
                    feat_tiles[b] = (ab, ftb)
                ab_sb, featTb = feat_tiles[b]

                # distance scores + self mask, accumulated in PSUM
                tk_ps = ptk.tile([128, N], f32, tag="tkps")
                lhsA = ab_sb[:, blk * 128:(blk + 1) * 128]
                for h in range(2):
                    nc.tensor.matmul(
                        tk_ps[:, h * 512:(h + 1) * 512],
                        lhsA,
                        ab_sb[:, N + h * 512:N + (h + 1) * 512],
                        start=True, stop=True,
                    )
                nc.tensor.matmul(
                    tk_ps[:, blk * 128:(blk + 1) * 128],
                    cbf_sb[:, 0:128],
                    cbf_sb[:, 128:256],
                    start=False, stop=True,
                    skip_group_check=True,
                )

                # top-16 per row
                if topk_f16:
                    tkh = spool.tile([128, N], f16, tag="tkh")
                    nc.scalar.activation(tkh[:], tk_ps[:], AF.Identity)
                    tksrc = tkh
                    vdt = f16
                else:
                    tksrc = tk_ps
                    vdt = f32
                vals = spool.tile([128, 16], vdt, tag="vals")
                idxp = spool.tile([128, 32], u16, tag="idxp")
                nc.vector.max(vals[:, 0:8], tksrc[:])
                nc.vector.max_index(idxp[:, 0:8], vals[:, 0:8], tksrc[:])
                nc.vector.match_replace(tksrc[:], vals[:, 0:8], tksrc[:], NEG)
                nc.vector.max(vals[:, 8:16], tksrc[:])
                nc.vector.max_index(idxp[:, 8:16], vals[:, 8:16], tksrc[:])
                # duplicate so each 32x32 transpose block carries two 16-row
                # replicas (dma_gather reads per-16-partition index copies)
                nc.vector.tensor_copy(idxp[:, 16:32], idxp[:, 0:16])

                itile = spool.tile([128, 128], u16, tag="itile")
                for t4 in range(4):
                    for pb in range(2):
                        nc.vector.transpose(
                            itile[32 * pb:32 * (pb + 1), 32 * t4:32 * (t4 + 1)],
                            idxp[32 * t4:32 * (t4 + 1), 0:32],
                        )
                nc.vector.tensor_copy(itile[64:128, :], itile[0:64, :])

                return itile

            def stage1b(b, blk, itile):
                nb = wpool.tile([128, 2048], f16, tag="nb")
                nc.gpsimd.dma_gather(
                    nb[:].rearrange("p (o g) -> p o g", o=1),
                    featrow_d[b],
                    itile[:].bitcast(i16),
                    2048, 2048, 128,
                    transpose=True,
                    single_packet=False,
                )
                return nb

            def stage2(b, blk, nb):
                featTb = feat_tiles[b][1]
                g2 = wpool.tile([128, 2048], f16, tag="g2")
                for c2 in range(2):
                    base = c2 * 1024
                    hs_pair = []
                    for h in range(2):
                        hp = ph1.tile([128, 1024], f32, tag="h1ps")
                        for q in range(2):
                            nc.tensor.matmul(
                                hp[:, q * 512:(q + 1) * 512],
                                w1a_sb[:, h * 128:(h + 1) * 128],
                                nb[0:64, base + q * 512:base + (q + 1) * 512],
                                start=True, stop=False,
                                skip_group_check=True,
                            )
                        for q in range(2):
                            r0 = blk * 128 + c2 * 64 + q * 32
                            ctr_rhs = featTb[:, r0:r0 + 32].to_broadcast([64, 32, 16])
                            nc.tensor.matmul(
                                hp[:, q * 512:(q + 1) * 512],
                                w1c_sb[:, h * 128:(h + 1) * 128],
                                ctr_rhs,
                                start=False, stop=True,
                                skip_group_check=True,
                            )
                        hs = spool.tile([128, 1024], f16, tag=f"h1sb{h}")
                        nc.scalar.activation(
                            hs[:], hp[:], act_fn, bias=b1_sb[:, h:h + 1]
                        )
                        hs_pair.append(hs)
                    p2 = pl2.tile([128, 1024], f32, tag="p2")
                    for h in range(2):
                        for q in range(2):
                            nc.tensor.matmul(
                                p2[:, q * 512:(q + 1) * 512],
                                w2_sb[:, h * 128:(h + 1) * 128],
                                hs_pair[h][:, q * 512:(q + 1) * 512],
                                start=(h == 0), stop=(h == 1),
                                skip_group_check=True,
                            )
                    nc.scalar.activation(
                        g2[:, base:base + 1024], p2[:], act_fn, bias=b2_sb[:, 0:1]
                    )

                red = spool.tile([128, 128], f16, tag="red")
                reng = nc.gpsimd if reduce_pool else nc.vector
                with nc.allow_low_precision(
                    reason="mean of 16 f16 gelu outputs; rel err ~5e-4 ok"
                ):
                    reng.tensor_reduce(
                        red[:], g2[:].rearrange("p (r k) -> p r k", k=K),
                        axis=mybir.AxisListType.X, op=mybir.AluOpType.add,
                    )
                outT = spool.tile([128, 128], f32, tag="outT")
                nc.scalar.activation(outT[:], red[:], AF.Identity, scale=1.0 / K)
                nc.sync.dma_start(out_d[b, blk], outT[:])

            # 3-stage software pipeline: iter i emits topk(i), gather(i-1),
            # MLP(i-2) so the Pool engine's gather desc-gen (the serial
            # bottleneck, ~16.5us/block) never waits on the DVE topk chain.
            nblocks = n_b * n_blk
            st1 = {}   # s -> (b, blk, itile)
            st1b = {}  # s -> (b, blk, nb)
            for i in range(nblocks + 2):
                if i < nblocks:
                    b, blk = divmod(i, n_blk)
                    st1[i] = (b, blk, stage1(b, blk))
                if i >= 1 and (i - 1) in st1:
                    b, blk, itile = st1.pop(i - 1)
                    st1b[i - 1] = (b, blk, stage1b(b, blk, itile))
                if i >= 2 and (i - 2) in st1b:
                    b, blk, nb = st1b.pop(i - 2)
                    stage2(b, blk, nb)

    nc.compile()
    return nc


def prep_core_inputs(points, features, W1, b1, W2, b2, core):
    """Host-side packing of one core's inputs (batches core*BL .. core*BL+BL)."""
    import ml_dtypes
    bf = ml_dtypes.bfloat16
    sl = slice(core * BL, (core + 1) * BL)
    pts = points[sl]           # [BL, N, 3]
    fts = features[sl]         # [BL, N, F]

    f16 = np.float16
    featTb = np.ascontiguousarray(fts.transpose(0, 2, 1)).astype(f16)  # [BL, 64, N]
    featrow = np.concatenate([fts, fts], axis=2).astype(f16)           # [BL, N, 128]

    r = (pts.astype(np.float64) ** 2).sum(-1).astype(np.float32)  # [BL, N]
    p_hi = pts.astype(bf).astype(np.float32)
    p_lo = (pts - p_hi).astype(bf).astype(np.float32)
    r_hi = r.astype(bf).astype(np.float32)
    r_lo = (r - r_hi).astype(bf).astype(np.float32)

    ab = np.zeros((BL, 2, 16, N), np.float32)
    # lhs rows (A) pair with rhs rows (B); Tk = 2 p_i . p_j - r_j
    ab[:, 0, 0:3] = 2.0 * p_hi.transpose(0, 2, 1)
    ab[:, 0, 3:6] = 2.0 * p_lo.transpose(0, 2, 1)
    ab[:, 0, 6:9] = 2.0 * p_hi.transpose(0, 2, 1)
    ab[:, 0, 9] = -1.0
    ab[:, 0, 10] = -1.0
    ab[:, 1, 0:3] = p_hi.transpose(0, 2, 1)
    ab[:, 1, 3:6] = p_hi.transpose(0, 2, 1)
    ab[:, 1, 6:9] = p_lo.transpose(0, 2, 1)
    ab[:, 1, 9] = r_hi
    ab[:, 1, 10] = r_lo
    ab = ab.astype(bf)

    w1a = W1[0:64].astype(f16)                      # [64, 256]
    w1c = (W1[64:128] - W1[0:64]).astype(f16)       # [64, 256]
    w2p = np.empty((128, 256), np.float32)
    w2p[:, 0:128] = W2[0:128]
    w2p[:, 128:256] = W2[128:256]
    w2p = w2p.astype(f16)
    b1p = np.ascontiguousarray(b1.reshape(2, 128).T).astype(np.float32)
    b2p = np.ascontiguousarray(b2.reshape(128, 1)).astype(np.float32)

    eye = np.eye(128, dtype=np.float32)
    cbf = np.concatenate([eye, -MASK * eye], axis=1).astype(bf)

    return {
        "ab": np.ascontiguousarray(ab),
        "featTb": featTb,
        "featrow": np.ascontiguousarray(featrow),
        "w1a": np.ascontiguousarray(w1a),
        "w1c": np.ascontiguousarray(w1c),
        "w2": w2p, "b1": b1p, "b2": b2p,
        "cbf": np.ascontiguousarray(cbf),
    }


_CACHED = {}


def kernel(points, features, W1, b1, W2, b2):
    from concourse import bass_utils

    points = np.asarray(points, np.float32)
    features = np.asarray(features, np.float32)
    W1 = np.asarray(W1, np.float32)
    b1 = np.asarray(b1, np.float32)
    W2 = np.asarray(W2, np.float32)
    b2 = np.asarray(b2, np.float32)

    if "nc" not in _CACHED:
        _CACHED["nc"] = build_program()
    nc = _CACHED["nc"]

    in_maps = [
        prep_core_inputs(points, features, W1, b1, W2, b2, c)
        for c in range(NCORES)
    ]
    res = bass_utils.run_bass_kernel_spmd(
        nc, in_maps, core_ids=list(range(NCORES))
    )
    outs = []
    for c in range(NCORES):
        o = res.results[c]["out"]          # [BL, NBLK, 128, 128] = [b, blk, P, r]
        outs.append(o.transpose(0, 1, 3, 2).reshape(BL, N, P))
    return np.concatenate(outs, axis=0)


# revision 16
# speedup vs baseline: 1.0053x; 1.0053x over previous
"""Trainium2 Bass kernel: LocalEmbeddingLayer (KNN -> gather -> 2-layer GELU MLP -> mean).

Full-input contract: kernel(**inputs) takes the unsharded inputs and returns the
full [B, N, P] output. Internally shards batch B=32 across 8 NeuronCores (pure
data parallel, 4 batch elements per core), runs one SPMD Bass program on all
cores, and concatenates the per-core outputs.

v2 design (vs the ap_gather baseline at 1.84ms):
  - The neighbor-feature gather is a SWDGE dma_gather(transpose=True) from a
    DRAM row table featrow[N, 128] f16 (256B rows, [feat|feat]); the DMA
    engines land it directly in [feat-on-partition, 2048] layout. This
    replaces the ~35us/block ap_gather ucode (which serialized the whole
    pipeline) with a ~2-4us DMA that overlaps compute.
  - The whole MLP runs in f16 (full PE rate like bf16, ~2^-11 element error;
    measured f32r matmuls run ~2.5 cycles/col on HW, f16 ~1).
  - The center-feature term of layer 1 is a second accumulating matmul whose
    rhs is a stride-0 broadcast AP over an SBUF f16 featT tile (16x free-dim
    repeat), so center features are never gathered or materialized.
  - Distance scores stay in the proven bf16 hi/lo split matmul (~fp32
    selection accuracy); top-16 runs on an f16 copy of the scores (DVE 16-bit
    2x mode) with the self-mask at -1e4.
  - Mean over k on the GPSIMD (Pool) engine ucode to unload the DVE.
"""

import numpy as np

B, N, DPOS, F, P, K = 32, 1024, 3, 64, 128, 16
NCORES = 8
BL = B // NCORES          # batches per core
NBLK = N // 128           # row blocks per batch
MASK = 1.0e4              # self-match mask depth (f16-safe)
NEG = -3.0e4              # match_replace purge value (f16-safe)

TOPK_F16 = False          # f16 scans measured no faster on DVE; f32 keeps accuracy
REDUCE_POOL = False       # GPSIMD tensor_reduce can't do free-axis (X) reduces


def build_program(topk_f16=TOPK_F16, reduce_pool=REDUCE_POOL, gelu=True,
                  n_b=BL, n_blk=NBLK):
    import concourse.bacc as bacc
    import concourse.mybir as mybir
    from concourse.tile import TileContext

    f32 = mybir.dt.float32
    f16 = mybir.dt.float16
    bf16 = mybir.dt.bfloat16
    u16 = mybir.dt.uint16
    i16 = mybir.dt.int16
    AF = mybir.ActivationFunctionType
    act_fn = AF.Gelu if gelu else AF.Identity

    nc = bacc.Bacc("TRN2", target_bir_lowering=False,
                   dynamic_dma_scratch_size=131072, num_swdge_queues=4)

    ab_d = nc.dram_tensor("ab", [n_b, 2, 16, N], bf16, kind="ExternalInput")
    featTb_d = nc.dram_tensor("featTb", [n_b, 64, N], f16, kind="ExternalInput")
    featrow_d = nc.dram_tensor("featrow", [n_b, N, 128], f16, kind="ExternalInput")
    w1a_d = nc.dram_tensor("w1a", [64, 256], f16, kind="ExternalInput")
    w1c_d = nc.dram_tensor("w1c", [64, 256], f16, kind="ExternalInput")
    w2_d = nc.dram_tensor("w2", [128, 256], f16, kind="ExternalInput")
    b1_d = nc.dram_tensor("b1", [128, 2], f32, kind="ExternalInput")
    b2_d = nc.dram_tensor("b2", [128, 1], f32, kind="ExternalInput")
    cbf_d = nc.dram_tensor("cbf", [128, 256], bf16, kind="ExternalInput")
    out_d = nc.dram_tensor("out", [n_b, n_blk, 128, 128], f32, kind="ExternalOutput")

    with TileContext(nc) as tc:
        with (
            tc.tile_pool(name="const", bufs=1) as cpool,
            tc.tile_pool(name="feat", bufs=2) as fpool,
            tc.tile_pool(name="work", bufs=3) as wpool,
            tc.tile_pool(name="small", bufs=3) as spool,
            tc.tile_pool(name="ps_tk", bufs=1, space="PSUM") as ptk,
            tc.tile_pool(name="ps_h1", bufs=2, space="PSUM") as ph1,
            tc.tile_pool(name="ps_l2", bufs=1, space="PSUM") as pl2,
        ):
            w1a_sb = cpool.tile([64, 256], f16)
            nc.sync.dma_start(w1a_sb[:], w1a_d[:])
            w1c_sb = cpool.tile([64, 256], f16)
            nc.sync.dma_start(w1c_sb[:], w1c_d[:])
            w2_sb = cpool.tile([128, 256], f16)
            nc.sync.dma_start(w2_sb[:], w2_d[:])
            b1_sb = cpool.tile([128, 2], f32)
            nc.sync.dma_start(b1_sb[:], b1_d[:])
            b2_sb = cpool.tile([128, 1], f32)
            nc.sync.dma_start(b2_sb[:], b2_d[:])
            cbf_sb = cpool.tile([128, 256], bf16)   # cols 0:128 I, 128:256 -MASK*I
            nc.sync.dma_start(cbf_sb[:], cbf_d[:])

            feat_tiles = {}

            def stage1(b, blk):
                if blk == 0:
                    ab = fpool.tile([16, 2 * N], bf16, tag="ab")
                    nc.sync.dma_start(
                        ab[:].rearrange("d (x n) -> d x n", x=2),
                        ab_d[b].rearrange("x d n -> d x n"),
                    )
                    ftb = fpool.tile([64, N], f16, tag="featTb")
                    nc.sync.dma_start(ftb[:], featTb_d[b])
                    feat_tiles[b] = (ab, ftb)
                ab_sb, featTb = feat_tiles[b]

                # distance scores + self mask, accumulated in PSUM
                tk_ps = ptk.tile([128, N], f32, tag="tkps")
                lhsA = ab_sb[:, blk * 128:(blk + 1) * 128]
                for h in range(2):
                    nc.tensor.matmul(
                        tk_ps[:, h * 512:(h + 1) * 512],
                        lhsA,
                        ab_sb[:, N + h * 512:N + (h + 1) * 512],
                        start=True, stop=True,
                    )
                nc.tensor.matmul(
                    tk_ps[:, blk * 128:(blk + 1) * 128],
                    cbf_sb[:, 0:128],
                    cbf_sb[:, 128:256],
                    start=False, stop=True,
                    skip_group_check=True,
                )

                # top-16 per row
                if topk_f16:
                    tkh = spool.tile([128, N], f16, tag="tkh")
                    nc.scalar.activation(tkh[:], tk_ps[:], AF.Identity)
                    tksrc = tkh
                    vdt = f16
                else:
                    tksrc = tk_ps
                    vdt = f32
                vals = spool.tile([128, 16], vdt, tag="vals")
                idxp = spool.tile([128, 32], u16, tag="idxp")
                nc.vector.max(vals[:, 0:8], tksrc[:])
                nc.vector.max_index(idxp[:, 0:8], vals[:, 0:8], tksrc[:])
                nc.vector.match_replace(tksrc[:], vals[:, 0:8], tksrc[:], NEG)
                nc.vector.max(vals[:, 8:16], tksrc[:])
                nc.vector.max_index(idxp[:, 8:16], vals[:, 8:16], tksrc[:])
                # duplicate so each 32x32 transpose block carries two 16-row
                # replicas (dma_gather reads per-16-partition index copies)
                nc.vector.tensor_copy(idxp[:, 16:32], idxp[:, 0:16])

                itile = spool.tile([128, 128], u16, tag="itile")
                for t4 in range(4):
                    for pb in range(2):
                        nc.vector.transpose(
                            itile[32 * pb:32 * (pb + 1), 32 * t4:32 * (t4 + 1)],
                            idxp[32 * t4:32 * (t4 + 1), 0:32],
                        )
                nc.vector.tensor_copy(itile[64:128, :], itile[0:64, :])

                return itile

            def stage1b(b, blk, itile, s):
                nb = wpool.tile([128, 2048], f16, tag="nb")
                nc.gpsimd.dma_gather(
                    nb[:].rearrange("p (o g) -> p o g", o=1),
                    featrow_d[b],
                    itile[:].bitcast(i16),
                    2048, 2048, 128,
                    transpose=True,
                    single_packet=False,
                    queue_num=s % 4,
                )
                return nb

            def stage2(b, blk, nb):
                featTb = feat_tiles[b][1]
                g2 = wpool.tile([128, 2048], f16, tag="g2")
                for c2 in range(2):
                    base = c2 * 1024
                    hs_pair = []
                    for h in range(2):
                        hp = ph1.tile([128, 1024], f32, tag="h1ps")
                        for q in range(2):
                            nc.tensor.matmul(
                                hp[:, q * 512:(q + 1) * 512],
                                w1a_sb[:, h * 128:(h + 1) * 128],
                                nb[0:64, base + q * 512:base + (q + 1) * 512],
                                start=True, stop=False,
                                skip_group_check=True,
                            )
                        for q in range(2):
                            r0 = blk * 128 + c2 * 64 + q * 32
                            ctr_rhs = featTb[:, r0:r0 + 32].to_broadcast([64, 32, 16])
                            nc.tensor.matmul(
                                hp[:, q * 512:(q + 1) * 512],
                                w1c_sb[:, h * 128:(h + 1) * 128],
                                ctr_rhs,
                                start=False, stop=True,
                                skip_group_check=True,
                            )
                        hs = spool.tile([128, 1024], f16, tag=f"h1sb{h}")
                        nc.scalar.activation(
                            hs[:], hp[:], act_fn, bias=b1_sb[:, h:h + 1]
                        )
                        hs_pair.append(hs)
                    p2 = pl2.tile([128, 1024], f32, tag="p2")
                    for h in range(2):
                        for q in range(2):
                            nc.tensor.matmul(
                                p2[:, q * 512:(q + 1) * 512],
                                w2_sb[:, h * 128:(h + 1) * 128],
                                hs_pair[h][:, q * 512:(q + 1) * 512],
                                start=(h == 0), stop=(h == 1),
                                skip_group_check=True,
                            )
                    nc.scalar.activation(
                        g2[:, base:base + 1024], p2[:], act_fn, bias=b2_sb[:, 0:1]
                    )

                red = spool.tile([128, 128], f16, tag="red")
                reng = nc.gpsimd if reduce_pool else nc.vector
                with nc.allow_low_precision(
                    reason="mean of 16 f16 gelu outputs; rel err ~5e-4 ok"
                ):
                    reng.tensor_reduce(
                        red[:], g2[:].rearrange("p (r k) -> p r k", k=K),
                        axis=mybir.AxisListType.X, op=mybir.AluOpType.add,
                    )
                outT = spool.tile([128, 128], f32, tag="outT")
                nc.scalar.activation(outT[:], red[:], AF.Identity, scale=1.0 / K)
                nc.sync.dma_start(out_d[b, blk], outT[:])

            # 3-stage software pipeline: iter i emits topk(i), gather(i-1),
            # MLP(i-2) so the Pool engine's gather desc-gen (the serial
            # bottleneck, ~16.5us/block) never waits on the DVE topk chain.
            nblocks = n_b * n_blk
            st1 = {}   # s -> (b, blk, itile)
            st1b = {}  # s -> (b, blk, nb)
            for i in range(nblocks + 2):
                if i < nblocks:
                    b, blk = divmod(i, n_blk)
                    st1[i] = (b, blk, stage1(b, blk))
                if i >= 1 and (i - 1) in st1:
                    b, blk, itile = st1.pop(i - 1)
                    st1b[i - 1] = (b, blk, stage1b(b, blk, itile, i - 1))
                if i >= 2 and (i - 2) in st1b:
                    b, blk, nb = st1b.pop(i - 2)
                    stage2(b, blk, nb)

    nc.compile()
    return nc


def prep_core_inputs(points, features, W1, b1, W2, b2, core):
    """Host-side packing of one core's inputs (batches core*BL .. core*BL+BL)."""
    import ml_dtypes
    bf = ml_dtypes.bfloat16
    sl = slice(core * BL, (core + 1) * BL)
    pts = points[sl]           # [BL, N, 3]
    fts = features[sl]         # [BL, N, F]

    f16 = np.float16
    featTb = np.ascontiguousarray(fts.transpose(0, 2, 1)).astype(f16)  # [BL, 64, N]
    featrow = np.concatenate([fts, fts], axis=2).astype(f16)           # [BL, N, 128]

    r = (pts.astype(np.float64) ** 2).sum(-1).astype(np.float32)  # [BL, N]
    p_hi = pts.astype(bf).astype(np.float32)
    p_lo = (pts - p_hi).astype(bf).astype(np.float32)
    r_hi = r.astype(bf).astype(np.float32)
    r_lo = (r - r_hi).astype(bf).astype(np.float32)

    ab = np.zeros((BL, 2, 16, N), np.float32)
    # lhs rows (A) pair with rhs rows (B); Tk = 2 p_i . p_j - r_j
    ab[:, 0, 0:3] = 2.0 * p_hi.transpose(0, 2, 1)
    ab[:, 0, 3:6] = 2.0 * p_lo.transpose(0, 2, 1)
    ab[:, 0, 6:9] = 2.0 * p_hi.transpose(0, 2, 1)
    ab[:, 0, 9] = -1.0
    ab[:, 0, 10] = -1.0
    ab[:, 1, 0:3] = p_hi.transpose(0, 2, 1)
    ab[:, 1, 3:6] = p_hi.transpose(0, 2, 1)
    ab[:, 1, 6:9] = p_lo.transpose(0, 2, 1)
    ab[:, 1, 9] = r_hi
    ab[:, 1, 10] = r_lo
    ab = ab.astype(bf)

    w1a = W1[0:64].astype(f16)                      # [64, 256]
    w1c = (W1[64:128] - W1[0:64]).astype(f16)       # [64, 256]
    w2p = np.empty((128, 256), np.float32)
    w2p[:, 0:128] = W2[0:128]
    w2p[:, 128:256] = W2[128:256]
    w2p = w2p.astype(f16)
    b1p = np.ascontiguousarray(b1.reshape(2, 128).T).astype(np.float32)
    b2p = np.ascontiguousarray(b2.reshape(128, 1)).astype(np.float32)

    eye = np.eye(128, dtype=np.float32)
    cbf = np.concatenate([eye, -MASK * eye], axis=1).astype(bf)

    return {
        "ab": np.ascontiguousarray(ab),
        "featTb": featTb,
        "featrow": np.ascontiguousarray(featrow),
        "w1a": np.ascontiguousarray(w1a),
        "w1c": np.ascontiguousarray(w1c),
        "w2": w2p, "b1": b1p, "b2": b2p,
        "cbf": np.ascontiguousarray(cbf),
    }


_CACHED = {}


def kernel(points, features, W1, b1, W2, b2):
    from concourse import bass_utils

    points = np.asarray(points, np.float32)
    features = np.asarray(features, np.float32)
    W1 = np.asarray(W1, np.float32)
    b1 = np.asarray(b1, np.float32)
    W2 = np.asarray(W2, np.float32)
    b2 = np.asarray(b2, np.float32)

    if "nc" not in _CACHED:
        _CACHED["nc"] = build_program()
    nc = _CACHED["nc"]

    in_maps = [
        prep_core_inputs(points, features, W1, b1, W2, b2, c)
        for c in range(NCORES)
    ]
    res = bass_utils.run_bass_kernel_spmd(
        nc, in_maps, core_ids=list(range(NCORES))
    )
    outs = []
    for c in range(NCORES):
        o = res.results[c]["out"]          # [BL, NBLK, 128, 128] = [b, blk, P, r]
        outs.append(o.transpose(0, 1, 3, 2).reshape(BL, N, P))
    return np.concatenate(outs, axis=0)


# revision 20
# speedup vs baseline: 1.0070x; 1.0017x over previous
"""Trainium2 Bass kernel: LocalEmbeddingLayer (KNN -> gather -> 2-layer GELU MLP -> mean).

Full-input contract: kernel(**inputs) takes the unsharded inputs and returns the
full [B, N, P] output. Internally shards batch B=32 across 8 NeuronCores (pure
data parallel, 4 batch elements per core), runs one SPMD Bass program on all
cores, and concatenates the per-core outputs.

v2 design (vs the ap_gather baseline at 1.84ms):
  - The neighbor-feature gather is a SWDGE dma_gather(transpose=True) from a
    DRAM row table featrow[N, 128] f16 (256B rows, [feat|feat]); the DMA
    engines land it directly in [feat-on-partition, 2048] layout. This
    replaces the ~35us/block ap_gather ucode (which serialized the whole
    pipeline) with a ~2-4us DMA that overlaps compute.
  - The whole MLP runs in f16 (full PE rate like bf16, ~2^-11 element error;
    measured f32r matmuls run ~2.5 cycles/col on HW, f16 ~1).
  - The center-feature term of layer 1 is a second accumulating matmul whose
    rhs is a stride-0 broadcast AP over an SBUF f16 featT tile (16x free-dim
    repeat), so center features are never gathered or materialized.
  - Distance scores stay in the proven bf16 hi/lo split matmul (~fp32
    selection accuracy); top-16 runs on an f16 copy of the scores (DVE 16-bit
    2x mode) with the self-mask at -1e4.
  - Mean over k on the GPSIMD (Pool) engine ucode to unload the DVE.
"""

import numpy as np

B, N, DPOS, F, P, K = 32, 1024, 3, 64, 128, 16
NCORES = 8
BL = B // NCORES          # batches per core
NBLK = N // 128           # row blocks per batch
MASK = 1.0e4              # self-match mask depth (f16-safe)
NEG = -3.0e4              # match_replace purge value (f16-safe)

TOPK_F16 = False          # f16 scans measured no faster on DVE; f32 keeps accuracy
REDUCE_POOL = False       # GPSIMD tensor_reduce can't do free-axis (X) reduces


def build_program(topk_f16=TOPK_F16, reduce_pool=REDUCE_POOL, gelu=True,
                  n_b=BL, n_blk=NBLK):
    import concourse.bacc as bacc
    import concourse.mybir as mybir
    from concourse.tile import TileContext

    f32 = mybir.dt.float32
    f16 = mybir.dt.float16
    bf16 = mybir.dt.bfloat16
    u16 = mybir.dt.uint16
    i16 = mybir.dt.int16
    AF = mybir.ActivationFunctionType
    act_fn = AF.Gelu if gelu else AF.Identity

    nc = bacc.Bacc("TRN2", target_bir_lowering=False,
                   dynamic_dma_scratch_size=131072, num_swdge_queues=4)

    ab_d = nc.dram_tensor("ab", [n_b, 2, 16, N], bf16, kind="ExternalInput")
    featTb_d = nc.dram_tensor("featTb", [n_b, 64, N], f16, kind="ExternalInput")
    featrow_d = nc.dram_tensor("featrow", [n_b, N, 128], f16, kind="ExternalInput")
    w1a_d = nc.dram_tensor("w1a", [64, 256], f16, kind="ExternalInput")
    w1c_d = nc.dram_tensor("w1c", [64, 256], f16, kind="ExternalInput")
    w2_d = nc.dram_tensor("w2", [128, 256], f16, kind="ExternalInput")
    b1_d = nc.dram_tensor("b1", [128, 2], f32, kind="ExternalInput")
    b2_d = nc.dram_tensor("b2", [128, 1], f32, kind="ExternalInput")
    cbf_d = nc.dram_tensor("cbf", [128, 256], bf16, kind="ExternalInput")
    out_d = nc.dram_tensor("out", [n_b, n_blk, 128, 128], f32, kind="ExternalOutput")

    with TileContext(nc) as tc:
        with (
            tc.tile_pool(name="const", bufs=1) as cpool,
            tc.tile_pool(name="feat", bufs=2) as fpool,
            tc.tile_pool(name="work", bufs=3) as wpool,
            tc.tile_pool(name="nbp", bufs=5) as nbpool,
            tc.tile_pool(name="idxt", bufs=6) as ipool,
            tc.tile_pool(name="small", bufs=3) as spool,
            tc.tile_pool(name="ps_tk", bufs=1, space="PSUM") as ptk,
            tc.tile_pool(name="ps_h1", bufs=2, space="PSUM") as ph1,
            tc.tile_pool(name="ps_l2", bufs=1, space="PSUM") as pl2,
        ):
            w1a_sb = cpool.tile([64, 256], f16)
            nc.sync.dma_start(w1a_sb[:], w1a_d[:])
            w1c_sb = cpool.tile([64, 256], f16)
            nc.sync.dma_start(w1c_sb[:], w1c_d[:])
            w2_sb = cpool.tile([128, 256], f16)
            nc.sync.dma_start(w2_sb[:], w2_d[:])
            b1_sb = cpool.tile([128, 2], f32)
            nc.sync.dma_start(b1_sb[:], b1_d[:])
            b2_sb = cpool.tile([128, 1], f32)
            nc.sync.dma_start(b2_sb[:], b2_d[:])
            cbf_sb = cpool.tile([128, 256], bf16)   # cols 0:128 I, 128:256 -MASK*I
            nc.sync.dma_start(cbf_sb[:], cbf_d[:])

            feat_tiles = {}

            def stage1(b, blk):
                if blk == 0:
                    ab = fpool.tile([16, 2 * N], bf16, tag="ab")
                    nc.sync.dma_start(
                        ab[:].rearrange("d (x n) -> d x n", x=2),
                        ab_d[b].rearrange("x d n -> d x n"),
                    )
                    ftb = fpool.tile([64, N], f16, tag="featTb")
                    nc.sync.dma_start(ftb[:], featTb_d[b])
                    feat_tiles[b] = (ab, ftb)
                ab_sb, featTb = feat_tiles[b]

                # distance scores + self mask, accumulated in PSUM
                tk_ps = ptk.tile([128, N], f32, tag="tkps")
                lhsA = ab_sb[:, blk * 128:(blk + 1) * 128]
                for h in range(2):
                    nc.tensor.matmul(
                        tk_ps[:, h * 512:(h + 1) * 512],
                        lhsA,
                        ab_sb[:, N + h * 512:N + (h + 1) * 512],
                        start=True, stop=True,
                    )
                nc.tensor.matmul(
                    tk_ps[:, blk * 128:(blk + 1) * 128],
                    cbf_sb[:, 0:128],
                    cbf_sb[:, 128:256],
                    start=False, stop=True,
                    skip_group_check=True,
                )

                # top-16 per row
                if topk_f16:
                    tkh = spool.tile([128, N], f16, tag="tkh")
                    nc.scalar.activation(tkh[:], tk_ps[:], AF.Identity)
                    tksrc = tkh
                    vdt = f16
                else:
                    tksrc = tk_ps
                    vdt = f32
                vals = ipool.tile([128, 16], vdt, tag="vals")
                idxp = ipool.tile([128, 32], u16, tag="idxp")
                nc.vector.max(vals[:, 0:8], tksrc[:])
                nc.vector.max_index(idxp[:, 0:8], vals[:, 0:8], tksrc[:])
                nc.vector.match_replace(tksrc[:], vals[:, 0:8], tksrc[:], NEG)
                nc.vector.max(vals[:, 8:16], tksrc[:])
                nc.vector.max_index(idxp[:, 8:16], vals[:, 8:16], tksrc[:])
                # duplicate so each 32x32 transpose block carries two 16-row
                # replicas (dma_gather reads per-16-partition index copies)
                nc.vector.tensor_copy(idxp[:, 16:32], idxp[:, 0:16])

                itile = ipool.tile([128, 128], u16, tag="itile")
                for t4 in range(4):
                    for pb in range(2):
                        nc.vector.transpose(
                            itile[32 * pb:32 * (pb + 1), 32 * t4:32 * (t4 + 1)],
                            idxp[32 * t4:32 * (t4 + 1), 0:32],
                        )
                nc.vector.tensor_copy(itile[64:128, :], itile[0:64, :])

                return itile

            def stage1b(b, blk, itile, s):
                nb = nbpool.tile([128, 2048], f16, tag="nb")
                nc.gpsimd.dma_gather(
                    nb[:].rearrange("p (o g) -> p o g", o=1),
                    featrow_d[b],
                    itile[:].bitcast(i16),
                    2048, 2048, 128,
                    transpose=True,
                    single_packet=False,
                    queue_num=s % 4,
                )
                return nb

            def stage2(b, blk, nb):
                featTb = feat_tiles[b][1]
                g2 = wpool.tile([128, 2048], f16, tag="g2")
                for c2 in range(2):
                    base = c2 * 1024
                    hs_pair = []
                    for h in range(2):
                        hp = ph1.tile([128, 1024], f32, tag="h1ps")
                        for q in range(2):
                            nc.tensor.matmul(
                                hp[:, q * 512:(q + 1) * 512],
                                w1a_sb[:, h * 128:(h + 1) * 128],
                                nb[0:64, base + q * 512:base + (q + 1) * 512],
                                start=True, stop=False,
                                skip_group_check=True,
                            )
                        for q in range(2):
                            r0 = blk * 128 + c2 * 64 + q * 32
                            ctr_rhs = featTb[:, r0:r0 + 32].to_broadcast([64, 32, 16])
                            nc.tensor.matmul(
                                hp[:, q * 512:(q + 1) * 512],
                                w1c_sb[:, h * 128:(h + 1) * 128],
                                ctr_rhs,
                                start=False, stop=True,
                                skip_group_check=True,
                            )
                        hs = spool.tile([128, 1024], f16, tag=f"h1sb{h}")
                        nc.scalar.activation(
                            hs[:], hp[:], act_fn, bias=b1_sb[:, h:h + 1]
                        )
                        hs_pair.append(hs)
                    p2 = pl2.tile([128, 1024], f32, tag="p2")
                    for h in range(2):
                        for q in range(2):
                            nc.tensor.matmul(
                                p2[:, q * 512:(q + 1) * 512],
                                w2_sb[:, h * 128:(h + 1) * 128],
                                hs_pair[h][:, q * 512:(q + 1) * 512],
                                start=(h == 0), stop=(h == 1),
                                skip_group_check=True,
                            )
                    nc.scalar.activation(
                        g2[:, base:base + 1024], p2[:], act_fn, bias=b2_sb[:, 0:1]
                    )

                red = spool.tile([128, 128], f16, tag="red")
                reng = nc.gpsimd if reduce_pool else nc.vector
                with nc.allow_low_precision(
                    reason="mean of 16 f16 gelu outputs; rel err ~5e-4 ok"
                ):
                    reng.tensor_reduce(
                        red[:], g2[:].rearrange("p (r k) -> p r k", k=K),
                        axis=mybir.AxisListType.X, op=mybir.AluOpType.add,
                    )
                outT = spool.tile([128, 128], f32, tag="outT")
                nc.scalar.activation(outT[:], red[:], AF.Identity, scale=1.0 / K)
                nc.sync.dma_start(out_d[b, blk], outT[:])

            # 3-stage software pipeline: iter i emits topk(i), gather(i-1),
            # MLP(i-2) so the Pool engine's gather desc-gen (the serial
            # bottleneck, ~16.5us/block) never waits on the DVE topk chain.
            nblocks = n_b * n_blk
            st1 = {}   # s -> (b, blk, itile)
            st1b = {}  # s -> (b, blk, nb)
            for i in range(nblocks + 2):
                if i < nblocks:
                    b, blk = divmod(i, n_blk)
                    st1[i] = (b, blk, stage1(b, blk))
                if i >= 1 and (i - 1) in st1:
                    b, blk, itile = st1.pop(i - 1)
                    st1b[i - 1] = (b, blk, stage1b(b, blk, itile, i - 1))
                if i >= 2 and (i - 2) in st1b:
                    b, blk, nb = st1b.pop(i - 2)
                    stage2(b, blk, nb)

    nc.compile()
    return nc


def prep_core_inputs(points, features, W1, b1, W2, b2, core):
    """Host-side packing of one core's inputs (batches core*BL .. core*BL+BL)."""
    import ml_dtypes
    bf = ml_dtypes.bfloat16
    sl = slice(core * BL, (core + 1) * BL)
    pts = points[sl]           # [BL, N, 3]
    fts = features[sl]         # [BL, N, F]

    f16 = np.float16
    featTb = np.ascontiguousarray(fts.transpose(0, 2, 1)).astype(f16)  # [BL, 64, N]
    featrow = np.concatenate([fts, fts], axis=2).astype(f16)           # [BL, N, 128]

    r = (pts.astype(np.float64) ** 2).sum(-1).astype(np.float32)  # [BL, N]
    p_hi = pts.astype(bf).astype(np.float32)
    p_lo = (pts - p_hi).astype(bf).astype(np.float32)
    r_hi = r.astype(bf).astype(np.float32)
    r_lo = (r - r_hi).astype(bf).astype(np.float32)

    ab = np.zeros((BL, 2, 16, N), np.float32)
    # lhs rows (A) pair with rhs rows (B); Tk = 2 p_i . p_j - r_j
    ab[:, 0, 0:3] = 2.0 * p_hi.transpose(0, 2, 1)
    ab[:, 0, 3:6] = 2.0 * p_lo.transpose(0, 2, 1)
    ab[:, 0, 6:9] = 2.0 * p_hi.transpose(0, 2, 1)
    ab[:, 0, 9] = -1.0
    ab[:, 0, 10] = -1.0
    ab[:, 1, 0:3] = p_hi.transpose(0, 2, 1)
    ab[:, 1, 3:6] = p_hi.transpose(0, 2, 1)
    ab[:, 1, 6:9] = p_lo.transpose(0, 2, 1)
    ab[:, 1, 9] = r_hi
    ab[:, 1, 10] = r_lo
    ab = ab.astype(bf)

    w1a = W1[0:64].astype(f16)                      # [64, 256]
    w1c = (W1[64:128] - W1[0:64]).astype(f16)       # [64, 256]
    w2p = np.empty((128, 256), np.float32)
    w2p[:, 0:128] = W2[0:128]
    w2p[:, 128:256] = W2[128:256]
    w2p = w2p.astype(f16)
    b1p = np.ascontiguousarray(b1.reshape(2, 128).T).astype(np.float32)
    b2p = np.ascontiguousarray(b2.reshape(128, 1)).astype(np.float32)

    eye = np.eye(128, dtype=np.float32)
    cbf = np.concatenate([eye, -MASK * eye], axis=1).astype(bf)

    return {
        "ab": np.ascontiguousarray(ab),
        "featTb": featTb,
        "featrow": np.ascontiguousarray(featrow),
        "w1a": np.ascontiguousarray(w1a),
        "w1c": np.ascontiguousarray(w1c),
        "w2": w2p, "b1": b1p, "b2": b2p,
        "cbf": np.ascontiguousarray(cbf),
    }


_CACHED = {}


def kernel(points, features, W1, b1, W2, b2):
    from concourse import bass_utils

    points = np.asarray(points, np.float32)
    features = np.asarray(features, np.float32)
    W1 = np.asarray(W1, np.float32)
    b1 = np.asarray(b1, np.float32)
    W2 = np.asarray(W2, np.float32)
    b2 = np.asarray(b2, np.float32)

    if "nc" not in _CACHED:
        _CACHED["nc"] = build_program()
    nc = _CACHED["nc"]

    in_maps = [
        prep_core_inputs(points, features, W1, b1, W2, b2, c)
        for c in range(NCORES)
    ]
    res = bass_utils.run_bass_kernel_spmd(
        nc, in_maps, core_ids=list(range(NCORES))
    )
    outs = []
    for c in range(NCORES):
        o = res.results[c]["out"]          # [BL, NBLK, 128, 128] = [b, blk, P, r]
        outs.append(o.transpose(0, 1, 3, 2).reshape(BL, N, P))
    return np.concatenate(outs, axis=0)


# revision 21
# speedup vs baseline: 1.3613x; 1.3518x over previous
"""Trainium2 Bass kernel: LocalEmbeddingLayer (KNN -> gather -> 2-layer GELU MLP -> mean).

Full-input contract: kernel(**inputs) takes the unsharded inputs and returns the
full [B, N, P] output. Internally shards batch B=32 across 8 NeuronCores (pure
data parallel, 4 batch elements per core), runs one SPMD Bass program on all
cores, and concatenates the per-core outputs.

v2 design (vs the ap_gather baseline at 1.84ms):
  - The neighbor-feature gather is a SWDGE dma_gather(transpose=True) from a
    DRAM row table featrow[N, 128] f16 (256B rows, [feat|feat]); the DMA
    engines land it directly in [feat-on-partition, 2048] layout. This
    replaces the ~35us/block ap_gather ucode (which serialized the whole
    pipeline) with a ~2-4us DMA that overlaps compute.
  - The whole MLP runs in f16 (full PE rate like bf16, ~2^-11 element error;
    measured f32r matmuls run ~2.5 cycles/col on HW, f16 ~1).
  - The center-feature term of layer 1 is a second accumulating matmul whose
    rhs is a stride-0 broadcast AP over an SBUF f16 featT tile (16x free-dim
    repeat), so center features are never gathered or materialized.
  - Distance scores stay in the proven bf16 hi/lo split matmul (~fp32
    selection accuracy); top-16 runs on an f16 copy of the scores (DVE 16-bit
    2x mode) with the self-mask at -1e4.
  - Mean over k on the GPSIMD (Pool) engine ucode to unload the DVE.
"""

import numpy as np

B, N, DPOS, F, P, K = 32, 1024, 3, 64, 128, 16
NCORES = 8
BL = B // NCORES          # batches per core
NBLK = N // 128           # row blocks per batch
MASK = 1.0e4              # self-match mask depth (f16-safe)
NEG = -3.0e4              # match_replace purge value (f16-safe)

TOPK_F16 = False          # f16 scans measured no faster on DVE; f32 keeps accuracy
REDUCE_POOL = False       # GPSIMD tensor_reduce can't do free-axis (X) reduces


def build_program(topk_f16=TOPK_F16, reduce_pool=REDUCE_POOL, gelu=True,
                  n_b=BL, n_blk=NBLK):
    import concourse.bacc as bacc
    import concourse.mybir as mybir
    from concourse.tile import TileContext

    f32 = mybir.dt.float32
    f16 = mybir.dt.float16
    bf16 = mybir.dt.bfloat16
    u16 = mybir.dt.uint16
    i16 = mybir.dt.int16
    AF = mybir.ActivationFunctionType
    act_fn = AF.Gelu if gelu else AF.Identity

    nc = bacc.Bacc("TRN2", target_bir_lowering=False,
                   dynamic_dma_scratch_size=131072, num_swdge_queues=4)

    ab_d = nc.dram_tensor("ab", [n_b, 2, 16, N], bf16, kind="ExternalInput")
    featTb_d = nc.dram_tensor("featTb", [n_b, 64, N], f16, kind="ExternalInput")
    featrow_d = nc.dram_tensor("featrow", [n_b, N, 128], f16, kind="ExternalInput")
    w1a_d = nc.dram_tensor("w1a", [64, 256], f16, kind="ExternalInput")
    w1c_d = nc.dram_tensor("w1c", [64, 256], f16, kind="ExternalInput")
    w2_d = nc.dram_tensor("w2", [128, 256], f16, kind="ExternalInput")
    b1_d = nc.dram_tensor("b1", [128, 2], f32, kind="ExternalInput")
    b2_d = nc.dram_tensor("b2", [128, 1], f32, kind="ExternalInput")
    cbf_d = nc.dram_tensor("cbf", [128, 256], bf16, kind="ExternalInput")
    out_d = nc.dram_tensor("out", [n_b, n_blk, 128, 128], f32, kind="ExternalOutput")

    with TileContext(nc) as tc:
        with (
            tc.tile_pool(name="const", bufs=1) as cpool,
            tc.tile_pool(name="feat", bufs=2) as fpool,
            tc.tile_pool(name="work", bufs=3) as wpool,
            tc.tile_pool(name="nbp", bufs=5) as nbpool,
            tc.tile_pool(name="idxt", bufs=6) as ipool,
            tc.tile_pool(name="small", bufs=3) as spool,
            tc.tile_pool(name="ps_tk", bufs=1, space="PSUM") as ptk,
            tc.tile_pool(name="ps_h1", bufs=2, space="PSUM") as ph1,
            tc.tile_pool(name="ps_l2", bufs=1, space="PSUM") as pl2,
        ):
            w1a_sb = cpool.tile([64, 256], f16)
            nc.sync.dma_start(w1a_sb[:], w1a_d[:])
            w1c_sb = cpool.tile([64, 256], f16)
            nc.sync.dma_start(w1c_sb[:], w1c_d[:])
            w2_sb = cpool.tile([128, 256], f16)
            nc.sync.dma_start(w2_sb[:], w2_d[:])
            b1_sb = cpool.tile([128, 2], f32)
            nc.sync.dma_start(b1_sb[:], b1_d[:])
            b2_sb = cpool.tile([128, 1], f32)
            nc.sync.dma_start(b2_sb[:], b2_d[:])
            cbf_sb = cpool.tile([128, 256], bf16)   # cols 0:128 I, 128:256 -MASK*I
            nc.sync.dma_start(cbf_sb[:], cbf_d[:])

            feat_tiles = {}

            def stage1(b, blk):
                if blk == 0:
                    ab = fpool.tile([16, 2 * N], bf16, tag="ab")
                    nc.sync.dma_start(
                        ab[:].rearrange("d (x n) -> d x n", x=2),
                        ab_d[b].rearrange("x d n -> d x n"),
                    )
                    ftb = fpool.tile([64, N], f16, tag="featTb")
                    nc.sync.dma_start(ftb[:], featTb_d[b])
                    feat_tiles[b] = (ab, ftb)
                ab_sb, featTb = feat_tiles[b]

                # distance scores + self mask, accumulated in PSUM
                tk_ps = ptk.tile([128, N], f32, tag="tkps")
                lhsA = ab_sb[:, blk * 128:(blk + 1) * 128]
                for h in range(2):
                    nc.tensor.matmul(
                        tk_ps[:, h * 512:(h + 1) * 512],
                        lhsA,
                        ab_sb[:, N + h * 512:N + (h + 1) * 512],
                        start=True, stop=True,
                    )
                nc.tensor.matmul(
                    tk_ps[:, blk * 128:(blk + 1) * 128],
                    cbf_sb[:, 0:128],
                    cbf_sb[:, 128:256],
                    start=False, stop=True,
                    skip_group_check=True,
                )

                # top-16 per row
                if topk_f16:
                    tkh = spool.tile([128, N], f16, tag="tkh")
                    nc.scalar.activation(tkh[:], tk_ps[:], AF.Identity)
                    tksrc = tkh
                    vdt = f16
                else:
                    tksrc = tk_ps
                    vdt = f32
                vals = ipool.tile([128, 16], vdt, tag="vals")
                idxp = ipool.tile([128, 32], u16, tag="idxp")
                nc.vector.max(vals[:, 0:8], tksrc[:])
                nc.vector.max_index(idxp[:, 0:8], vals[:, 0:8], tksrc[:])
                nc.vector.match_replace(tksrc[:], vals[:, 0:8], tksrc[:], NEG)
                nc.vector.max(vals[:, 8:16], tksrc[:])
                nc.vector.max_index(idxp[:, 8:16], vals[:, 8:16], tksrc[:])
                # duplicate so each 32x32 transpose block carries two 16-row
                # replicas (dma_gather reads per-16-partition index copies)
                nc.vector.tensor_copy(idxp[:, 16:32], idxp[:, 0:16])

                itile = ipool.tile([128, 128], u16, tag="itile")
                for t4 in range(4):
                    for pb in range(2):
                        nc.vector.transpose(
                            itile[32 * pb:32 * (pb + 1), 32 * t4:32 * (t4 + 1)],
                            idxp[32 * t4:32 * (t4 + 1), 0:32],
                        )
                nc.vector.tensor_copy(itile[64:128, :], itile[0:64, :])

                return itile

            def stage1b(b, blk, itile, s):
                nb = nbpool.tile([128, 2048], f16, tag="nb")
                # 4x512 single-packet chunks: batched descriptors (16 idx/desc)
                # keep the Q7's SBUF ring traffic low, rotating SWDGE queues
                # so chunk transfers overlap the next chunk's desc-gen
                for q in range(4):
                    nc.gpsimd.dma_gather(
                        nb[:, q * 512:(q + 1) * 512].rearrange(
                            "p (o g) -> p o g", o=1),
                        featrow_d[b],
                        itile[:, q * 32:(q + 1) * 32].bitcast(i16),
                        512, 512, 128,
                        transpose=True,
                        queue_num=(s * 4 + q) % 4,
                    )
                return nb

            def stage2(b, blk, nb):
                featTb = feat_tiles[b][1]
                g2 = wpool.tile([128, 2048], f16, tag="g2")
                for c2 in range(2):
                    base = c2 * 1024
                    hs_pair = []
                    for h in range(2):
                        hp = ph1.tile([128, 1024], f32, tag="h1ps")
                        for q in range(2):
                            nc.tensor.matmul(
                                hp[:, q * 512:(q + 1) * 512],
                                w1a_sb[:, h * 128:(h + 1) * 128],
                                nb[0:64, base + q * 512:base + (q + 1) * 512],
                                start=True, stop=False,
                                skip_group_check=True,
                            )
                        for q in range(2):
                            r0 = blk * 128 + c2 * 64 + q * 32
                            ctr_rhs = featTb[:, r0:r0 + 32].to_broadcast([64, 32, 16])
                            nc.tensor.matmul(
                                hp[:, q * 512:(q + 1) * 512],
                                w1c_sb[:, h * 128:(h + 1) * 128],
                                ctr_rhs,
                                start=False, stop=True,
                                skip_group_check=True,
                            )
                        hs = spool.tile([128, 1024], f16, tag=f"h1sb{h}")
                        nc.scalar.activation(
                            hs[:], hp[:], act_fn, bias=b1_sb[:, h:h + 1]
                        )
                        hs_pair.append(hs)
                    p2 = pl2.tile([128, 1024], f32, tag="p2")
                    for h in range(2):
                        for q in range(2):
                            nc.tensor.matmul(
                                p2[:, q * 512:(q + 1) * 512],
                                w2_sb[:, h * 128:(h + 1) * 128],
                                hs_pair[h][:, q * 512:(q + 1) * 512],
                                start=(h == 0), stop=(h == 1),
                                skip_group_check=True,
                            )
                    nc.scalar.activation(
                        g2[:, base:base + 1024], p2[:], act_fn, bias=b2_sb[:, 0:1]
                    )

                red = spool.tile([128, 128], f16, tag="red")
                reng = nc.gpsimd if reduce_pool else nc.vector
                with nc.allow_low_precision(
                    reason="mean of 16 f16 gelu outputs; rel err ~5e-4 ok"
                ):
                    reng.tensor_reduce(
                        red[:], g2[:].rearrange("p (r k) -> p r k", k=K),
                        axis=mybir.AxisListType.X, op=mybir.AluOpType.add,
                    )
                outT = spool.tile([128, 128], f32, tag="outT")
                nc.scalar.activation(outT[:], red[:], AF.Identity, scale=1.0 / K)
                nc.sync.dma_start(out_d[b, blk], outT[:])

            # 3-stage software pipeline: iter i emits topk(i), gather(i-1),
            # MLP(i-2) so the Pool engine's gather desc-gen (the serial
            # bottleneck, ~16.5us/block) never waits on the DVE topk chain.
            nblocks = n_b * n_blk
            st1 = {}   # s -> (b, blk, itile)
            st1b = {}  # s -> (b, blk, nb)
            for i in range(nblocks + 2):
                if i < nblocks:
                    b, blk = divmod(i, n_blk)
                    st1[i] = (b, blk, stage1(b, blk))
                if i >= 1 and (i - 1) in st1:
                    b, blk, itile = st1.pop(i - 1)
                    st1b[i - 1] = (b, blk, stage1b(b, blk, itile, i - 1))
                if i >= 2 and (i - 2) in st1b:
                    b, blk, nb = st1b.pop(i - 2)
                    stage2(b, blk, nb)

    nc.compile()
    return nc


def prep_core_inputs(points, features, W1, b1, W2, b2, core):
    """Host-side packing of one core's inputs (batches core*BL .. core*BL+BL)."""
    import ml_dtypes
    bf = ml_dtypes.bfloat16
    sl = slice(core * BL, (core + 1) * BL)
    pts = points[sl]           # [BL, N, 3]
    fts = features[sl]         # [BL, N, F]

    f16 = np.float16
    featTb = np.ascontiguousarray(fts.transpose(0, 2, 1)).astype(f16)  # [BL, 64, N]
    featrow = np.concatenate([fts, fts], axis=2).astype(f16)           # [BL, N, 128]

    r = (pts.astype(np.float64) ** 2).sum(-1).astype(np.float32)  # [BL, N]
    p_hi = pts.astype(bf).astype(np.float32)
    p_lo = (pts - p_hi).astype(bf).astype(np.float32)
    r_hi = r.astype(bf).astype(np.float32)
    r_lo = (r - r_hi).astype(bf).astype(np.float32)

    ab = np.zeros((BL, 2, 16, N), np.float32)
    # lhs rows (A) pair with rhs rows (B); Tk = 2 p_i . p_j - r_j
    ab[:, 0, 0:3] = 2.0 * p_hi.transpose(0, 2, 1)
    ab[:, 0, 3:6] = 2.0 * p_lo.transpose(0, 2, 1)
    ab[:, 0, 6:9] = 2.0 * p_hi.transpose(0, 2, 1)
    ab[:, 0, 9] = -1.0
    ab[:, 0, 10] = -1.0
    ab[:, 1, 0:3] = p_hi.transpose(0, 2, 1)
    ab[:, 1, 3:6] = p_hi.transpose(0, 2, 1)
    ab[:, 1, 6:9] = p_lo.transpose(0, 2, 1)
    ab[:, 1, 9] = r_hi
    ab[:, 1, 10] = r_lo
    ab = ab.astype(bf)

    w1a = W1[0:64].astype(f16)                      # [64, 256]
    w1c = (W1[64:128] - W1[0:64]).astype(f16)       # [64, 256]
    w2p = np.empty((128, 256), np.float32)
    w2p[:, 0:128] = W2[0:128]
    w2p[:, 128:256] = W2[128:256]
    w2p = w2p.astype(f16)
    b1p = np.ascontiguousarray(b1.reshape(2, 128).T).astype(np.float32)
    b2p = np.ascontiguousarray(b2.reshape(128, 1)).astype(np.float32)

    eye = np.eye(128, dtype=np.float32)
    cbf = np.concatenate([eye, -MASK * eye], axis=1).astype(bf)

    return {
        "ab": np.ascontiguousarray(ab),
        "featTb": featTb,
        "featrow": np.ascontiguousarray(featrow),
        "w1a": np.ascontiguousarray(w1a),
        "w1c": np.ascontiguousarray(w1c),
        "w2": w2p, "b1": b1p, "b2": b2p,
        "cbf": np.ascontiguousarray(cbf),
    }


_CACHED = {}


def kernel(points, features, W1, b1, W2, b2):
    from concourse import bass_utils

    points = np.asarray(points, np.float32)
    features = np.asarray(features, np.float32)
    W1 = np.asarray(W1, np.float32)
    b1 = np.asarray(b1, np.float32)
    W2 = np.asarray(W2, np.float32)
    b2 = np.asarray(b2, np.float32)

    if "nc" not in _CACHED:
        _CACHED["nc"] = build_program()
    nc = _CACHED["nc"]

    in_maps = [
        prep_core_inputs(points, features, W1, b1, W2, b2, c)
        for c in range(NCORES)
    ]
    res = bass_utils.run_bass_kernel_spmd(
        nc, in_maps, core_ids=list(range(NCORES))
    )
    outs = []
    for c in range(NCORES):
        o = res.results[c]["out"]          # [BL, NBLK, 128, 128] = [b, blk, P, r]
        outs.append(o.transpose(0, 1, 3, 2).reshape(BL, N, P))
    return np.concatenate(outs, axis=0)


# revision 23
# speedup vs baseline: 2.1070x; 1.5478x over previous
"""Trainium2 Bass kernel: LocalEmbeddingLayer (KNN -> gather -> 2-layer GELU MLP -> mean).

Full-input contract: kernel(**inputs) takes the unsharded inputs and returns the
full [B, N, P] output. Internally shards batch B=32 across 8 NeuronCores (pure
data parallel, 4 batch elements per core), runs one SPMD Bass program on all
cores, and concatenates the per-core outputs.

v2 design (vs the ap_gather baseline at 1.84ms):
  - The neighbor-feature gather is a SWDGE dma_gather(transpose=True) from a
    DRAM row table featrow[N, 128] f16 (256B rows, [feat|feat]); the DMA
    engines land it directly in [feat-on-partition, 2048] layout. This
    replaces the ~35us/block ap_gather ucode (which serialized the whole
    pipeline) with a ~2-4us DMA that overlaps compute.
  - The whole MLP runs in f16 (full PE rate like bf16, ~2^-11 element error;
    measured f32r matmuls run ~2.5 cycles/col on HW, f16 ~1).
  - The center-feature term of layer 1 is a second accumulating matmul whose
    rhs is a stride-0 broadcast AP over an SBUF f16 featT tile (16x free-dim
    repeat), so center features are never gathered or materialized.
  - Distance scores stay in the proven bf16 hi/lo split matmul (~fp32
    selection accuracy); top-16 runs on an f16 copy of the scores (DVE 16-bit
    2x mode) with the self-mask at -1e4.
  - Mean over k on the GPSIMD (Pool) engine ucode to unload the DVE.
"""

import numpy as np

B, N, DPOS, F, P, K = 32, 1024, 3, 64, 128, 16
NCORES = 8
BL = B // NCORES          # batches per core
NBLK = N // 128           # row blocks per batch
MASK = 1.0e4              # self-match mask depth (f16-safe)
NEG = -3.0e4              # match_replace purge value (f16-safe)

TOPK_F16 = True           # frees the Tk PSUM bank early (Act casts to SBUF f16)
REDUCE_POOL = False       # GPSIMD tensor_reduce can't do free-axis (X) reduces


def build_program(topk_f16=TOPK_F16, reduce_pool=REDUCE_POOL, gelu=True,
                  n_b=BL, n_blk=NBLK):
    import concourse.bacc as bacc
    import concourse.mybir as mybir
    from concourse.tile import TileContext

    f32 = mybir.dt.float32
    f16 = mybir.dt.float16
    bf16 = mybir.dt.bfloat16
    u16 = mybir.dt.uint16
    i16 = mybir.dt.int16
    AF = mybir.ActivationFunctionType
    act_fn = AF.Gelu if gelu else AF.Identity

    nc = bacc.Bacc("TRN2", target_bir_lowering=False,
                   dynamic_dma_scratch_size=131072, num_swdge_queues=4)

    ab_d = nc.dram_tensor("ab", [n_b, 2, 16, N], bf16, kind="ExternalInput")
    featTb_d = nc.dram_tensor("featTb", [n_b, 64, N], f16, kind="ExternalInput")
    featrow_d = nc.dram_tensor("featrow", [n_b, N, 128], f16, kind="ExternalInput")
    w1a_d = nc.dram_tensor("w1a", [64, 256], f16, kind="ExternalInput")
    w1c_d = nc.dram_tensor("w1c", [64, 256], f16, kind="ExternalInput")
    w2_d = nc.dram_tensor("w2", [128, 256], f16, kind="ExternalInput")
    b1_d = nc.dram_tensor("b1", [128, 2], f32, kind="ExternalInput")
    b2_d = nc.dram_tensor("b2", [128, 1], f32, kind="ExternalInput")
    cbf_d = nc.dram_tensor("cbf", [128, 256], bf16, kind="ExternalInput")
    out_d = nc.dram_tensor("out", [n_b, n_blk, 128, 128], f32, kind="ExternalOutput")

    with TileContext(nc) as tc:
        with (
            tc.tile_pool(name="const", bufs=1) as cpool,
            tc.tile_pool(name="feat", bufs=2) as fpool,
            tc.tile_pool(name="work", bufs=3) as wpool,
            tc.tile_pool(name="nbp", bufs=5) as nbpool,
            tc.tile_pool(name="idxt", bufs=6) as ipool,
            tc.tile_pool(name="small", bufs=3) as spool,
            tc.tile_pool(name="ps_tk", bufs=1, space="PSUM") as ptk,
            tc.tile_pool(name="ps_h1", bufs=2, space="PSUM") as ph1,
            tc.tile_pool(name="ps_l2", bufs=2, space="PSUM") as pl2,
        ):
            w1a_sb = cpool.tile([64, 256], f16)
            nc.sync.dma_start(w1a_sb[:], w1a_d[:])
            w1c_sb = cpool.tile([64, 256], f16)
            nc.sync.dma_start(w1c_sb[:], w1c_d[:])
            w2_sb = cpool.tile([128, 256], f16)
            nc.sync.dma_start(w2_sb[:], w2_d[:])
            b1_sb = cpool.tile([128, 2], f32)
            nc.sync.dma_start(b1_sb[:], b1_d[:])
            b2_sb = cpool.tile([128, 1], f32)
            nc.sync.dma_start(b2_sb[:], b2_d[:])
            cbf_sb = cpool.tile([128, 256], bf16)   # cols 0:128 I, 128:256 -MASK*I
            nc.sync.dma_start(cbf_sb[:], cbf_d[:])

            feat_tiles = {}

            def stage1(b, blk):
                if blk == 0:
                    ab = fpool.tile([16, 2 * N], bf16, tag="ab")
                    nc.sync.dma_start(
                        ab[:].rearrange("d (x n) -> d x n", x=2),
                        ab_d[b].rearrange("x d n -> d x n"),
                    )
                    ftb = fpool.tile([64, N], f16, tag="featTb")
                    nc.sync.dma_start(ftb[:], featTb_d[b])
                    feat_tiles[b] = (ab, ftb)
                ab_sb, featTb = feat_tiles[b]

                # distance scores + self mask, accumulated in PSUM
                tk_ps = ptk.tile([128, N], f32, tag="tkps")
                lhsA = ab_sb[:, blk * 128:(blk + 1) * 128]
                for h in range(2):
                    nc.tensor.matmul(
                        tk_ps[:, h * 512:(h + 1) * 512],
                        lhsA,
                        ab_sb[:, N + h * 512:N + (h + 1) * 512],
                        start=True, stop=True,
                    )
                nc.tensor.matmul(
                    tk_ps[:, blk * 128:(blk + 1) * 128],
                    cbf_sb[:, 0:128],
                    cbf_sb[:, 128:256],
                    start=False, stop=True,
                    skip_group_check=True,
                )

                # top-16 per row
                if topk_f16:
                    tkh = spool.tile([128, N], f16, tag="tkh")
                    nc.scalar.activation(tkh[:], tk_ps[:], AF.Identity)
                    tksrc = tkh
                    vdt = f16
                else:
                    tksrc = tk_ps
                    vdt = f32
                vals = ipool.tile([128, 16], vdt, tag="vals")
                idxp = ipool.tile([128, 32], u16, tag="idxp")
                nc.vector.max(vals[:, 0:8], tksrc[:])
                nc.vector.max_index(idxp[:, 0:8], vals[:, 0:8], tksrc[:])
                nc.vector.match_replace(tksrc[:], vals[:, 0:8], tksrc[:], NEG)
                nc.vector.max(vals[:, 8:16], tksrc[:])
                nc.vector.max_index(idxp[:, 8:16], vals[:, 8:16], tksrc[:])
                # duplicate so each 32x32 transpose block carries two 16-row
                # replicas (dma_gather reads per-16-partition index copies)
                nc.vector.tensor_copy(idxp[:, 16:32], idxp[:, 0:16])

                itile = ipool.tile([128, 128], u16, tag="itile")
                for t4 in range(4):
                    for pb in range(2):
                        nc.vector.transpose(
                            itile[32 * pb:32 * (pb + 1), 32 * t4:32 * (t4 + 1)],
                            idxp[32 * t4:32 * (t4 + 1), 0:32],
                        )
                nc.vector.tensor_copy(itile[64:128, :], itile[0:64, :])

                return itile

            def stage1b(b, blk, itile, s):
                nb = nbpool.tile([128, 2048], f16, tag="nb")
                # 4x512 single-packet chunks: batched descriptors (16 idx/desc)
                # keep the Q7's SBUF ring traffic low, rotating SWDGE queues
                # so chunk transfers overlap the next chunk's desc-gen
                for q in range(4):
                    nc.gpsimd.dma_gather(
                        nb[:, q * 512:(q + 1) * 512].rearrange(
                            "p (o g) -> p o g", o=1),
                        featrow_d[b],
                        itile[:, q * 32:(q + 1) * 32].bitcast(i16),
                        512, 512, 128,
                        transpose=True,
                        queue_num=(s * 4 + q) % 4,
                    )
                return nb

            def stage2(b, blk, nb):
                featTb = feat_tiles[b][1]
                g2 = wpool.tile([128, 2048], f16, tag="g2")
                for c2 in range(2):
                    base = c2 * 1024
                    hs_pair = []
                    for h in range(2):
                        hp = ph1.tile([128, 1024], f32, tag="h1ps")
                        for q in range(2):
                            nc.tensor.matmul(
                                hp[:, q * 512:(q + 1) * 512],
                                w1a_sb[:, h * 128:(h + 1) * 128],
                                nb[0:64, base + q * 512:base + (q + 1) * 512],
                                start=True, stop=False,
                                skip_group_check=True,
                            )
                        for q in range(2):
                            r0 = blk * 128 + c2 * 64 + q * 32
                            ctr_rhs = featTb[:, r0:r0 + 32].to_broadcast([64, 32, 16])
                            nc.tensor.matmul(
                                hp[:, q * 512:(q + 1) * 512],
                                w1c_sb[:, h * 128:(h + 1) * 128],
                                ctr_rhs,
                                start=False, stop=True,
                                skip_group_check=True,
                            )
                        hs = spool.tile([128, 1024], f16, tag=f"h1sb{h}")
                        nc.scalar.activation(
                            hs[:], hp[:], act_fn, bias=b1_sb[:, h:h + 1]
                        )
                        hs_pair.append(hs)
                    for q in range(2):
                        p2 = pl2.tile([128, 512], f32, tag="p2")
                        for h in range(2):
                            nc.tensor.matmul(
                                p2[:],
                                w2_sb[:, h * 128:(h + 1) * 128],
                                hs_pair[h][:, q * 512:(q + 1) * 512],
                                start=(h == 0), stop=(h == 1),
                                skip_group_check=True,
                            )
                        nc.scalar.activation(
                            g2[:, base + q * 512:base + (q + 1) * 512], p2[:],
                            act_fn, bias=b2_sb[:, 0:1]
                        )

                red = spool.tile([128, 128], f16, tag="red")
                reng = nc.gpsimd if reduce_pool else nc.vector
                with nc.allow_low_precision(
                    reason="mean of 16 f16 gelu outputs; rel err ~5e-4 ok"
                ):
                    reng.tensor_reduce(
                        red[:], g2[:].rearrange("p (r k) -> p r k", k=K),
                        axis=mybir.AxisListType.X, op=mybir.AluOpType.add,
                    )
                outT = spool.tile([128, 128], f32, tag="outT")
                nc.scalar.activation(outT[:], red[:], AF.Identity, scale=1.0 / K)
                nc.sync.dma_start(out_d[b, blk], outT[:])

            # 3-stage software pipeline: iter i emits topk(i), gather(i-1),
            # MLP(i-2) so the Pool engine's gather desc-gen (the serial
            # bottleneck, ~16.5us/block) never waits on the DVE topk chain.
            nblocks = n_b * n_blk
            st1 = {}   # s -> (b, blk, itile)
            st1b = {}  # s -> (b, blk, nb)
            for i in range(nblocks + 2):
                if i < nblocks:
                    b, blk = divmod(i, n_blk)
                    st1[i] = (b, blk, stage1(b, blk))
                if i >= 1 and (i - 1) in st1:
                    b, blk, itile = st1.pop(i - 1)
                    st1b[i - 1] = (b, blk, stage1b(b, blk, itile, i - 1))
                if i >= 2 and (i - 2) in st1b:
                    b, blk, nb = st1b.pop(i - 2)
                    stage2(b, blk, nb)

    nc.compile()
    return nc


def prep_core_inputs(points, features, W1, b1, W2, b2, core):
    """Host-side packing of one core's inputs (batches core*BL .. core*BL+BL)."""
    import ml_dtypes
    bf = ml_dtypes.bfloat16
    sl = slice(core * BL, (core + 1) * BL)
    pts = points[sl]           # [BL, N, 3]
    fts = features[sl]         # [BL, N, F]

    f16 = np.float16
    featTb = np.ascontiguousarray(fts.transpose(0, 2, 1)).astype(f16)  # [BL, 64, N]
    featrow = np.concatenate([fts, fts], axis=2).astype(f16)           # [BL, N, 128]

    r = (pts.astype(np.float64) ** 2).sum(-1).astype(np.float32)  # [BL, N]
    p_hi = pts.astype(bf).astype(np.float32)
    p_lo = (pts - p_hi).astype(bf).astype(np.float32)
    r_hi = r.astype(bf).astype(np.float32)
    r_lo = (r - r_hi).astype(bf).astype(np.float32)

    ab = np.zeros((BL, 2, 16, N), np.float32)
    # lhs rows (A) pair with rhs rows (B); Tk = 2 p_i . p_j - r_j
    ab[:, 0, 0:3] = 2.0 * p_hi.transpose(0, 2, 1)
    ab[:, 0, 3:6] = 2.0 * p_lo.transpose(0, 2, 1)
    ab[:, 0, 6:9] = 2.0 * p_hi.transpose(0, 2, 1)
    ab[:, 0, 9] = -1.0
    ab[:, 0, 10] = -1.0
    ab[:, 1, 0:3] = p_hi.transpose(0, 2, 1)
    ab[:, 1, 3:6] = p_hi.transpose(0, 2, 1)
    ab[:, 1, 6:9] = p_lo.transpose(0, 2, 1)
    ab[:, 1, 9] = r_hi
    ab[:, 1, 10] = r_lo
    ab = ab.astype(bf)

    w1a = W1[0:64].astype(f16)                      # [64, 256]
    w1c = (W1[64:128] - W1[0:64]).astype(f16)       # [64, 256]
    w2p = np.empty((128, 256), np.float32)
    w2p[:, 0:128] = W2[0:128]
    w2p[:, 128:256] = W2[128:256]
    w2p = w2p.astype(f16)
    b1p = np.ascontiguousarray(b1.reshape(2, 128).T).astype(np.float32)
    b2p = np.ascontiguousarray(b2.reshape(128, 1)).astype(np.float32)

    eye = np.eye(128, dtype=np.float32)
    cbf = np.concatenate([eye, -MASK * eye], axis=1).astype(bf)

    return {
        "ab": np.ascontiguousarray(ab),
        "featTb": featTb,
        "featrow": np.ascontiguousarray(featrow),
        "w1a": np.ascontiguousarray(w1a),
        "w1c": np.ascontiguousarray(w1c),
        "w2": w2p, "b1": b1p, "b2": b2p,
        "cbf": np.ascontiguousarray(cbf),
    }


_CACHED = {}


def kernel(points, features, W1, b1, W2, b2):
    from concourse import bass_utils

    points = np.asarray(points, np.float32)
    features = np.asarray(features, np.float32)
    W1 = np.asarray(W1, np.float32)
    b1 = np.asarray(b1, np.float32)
    W2 = np.asarray(W2, np.float32)
    b2 = np.asarray(b2, np.float32)

    if "nc" not in _CACHED:
        _CACHED["nc"] = build_program()
    nc = _CACHED["nc"]

    in_maps = [
        prep_core_inputs(points, features, W1, b1, W2, b2, c)
        for c in range(NCORES)
    ]
    res = bass_utils.run_bass_kernel_spmd(
        nc, in_maps, core_ids=list(range(NCORES))
    )
    outs = []
    for c in range(NCORES):
        o = res.results[c]["out"]          # [BL, NBLK, 128, 128] = [b, blk, P, r]
        outs.append(o.transpose(0, 1, 3, 2).reshape(BL, N, P))
    return np.concatenate(outs, axis=0)
